# revision 65
# baseline (speedup 1.0000x reference)
"""Trainium2 Bass kernel for nn_Attention_39651138076722.

ChannelLayerNorm -> qkv 1x1 conv -> 4-head spatial attention (N=4096, dh=32)
-> proj 1x1 conv -> residual.   B=4, C=128, H=W=64.

Sharding: 8 cores = 4 batches x 2 head-pairs. Each core computes the partial
proj output of its 2 heads for its batch; the host sums the two partials.
LayerNorm affine (norm_w/norm_b) is folded into the qkv weights on the host.
Big matmuls run as float32r (~1.6e-4 max rel err, 4x faster than fp32).

Attention works on S^T = k.T q tiles [m=128, n=512]: four K=32 matmuls are
row-packed into the PE array per group (2 heads x 2 m-chunks, via
row-duplicated qq2/kk2 layouts), one big exp on ACT per 3-bank PSUM group,
and PV accumulates h rows + a ones-row (softmax denominator) per head.

Engine balance: ACT is the bottleneck (exp over 2*N^2 elements/core), so the
LN rsqrt runs as exp(-0.5*ln(var+eps)) -- Ln and Exp share one ACT table set,
so no table reloads ever occur and LN work interleaves freely with attention
exps.  x^2 and the softmax-denominator partition-broadcast run on the
otherwise-idle GPSIMD/Pool engine.  The old DRAM round-trip for broadcasting
the per-position 1/denominator is gone.
"""
import sys
sys.path.insert(0, "/opt/trn_rl_repo")

import numpy as np
import concourse.bass as bass
import concourse.bass_isa as bass_isa
import concourse.tile as tile
from concourse import bacc, mybir
from concourse.bass_utils import run_bass_kernel_spmd

F32 = mybir.dt.float32
F32R = mybir.dt.float32r
BF16 = mybir.dt.bfloat16
AF = mybir.ActivationFunctionType
OP = mybir.AluOpType

# The act-table insertion pass assigns each activation func the FIRST table
# set containing it (Ln -> natural_log, Exp -> exp_and_others), which thrashes
# table loads between the LN-rsqrt (ln,exp) and attention exps.  Keep only
# natural_log_exp_and_others (holds ln AND exp) so everything shares one set
# and only a single initial load is emitted.  Positions are preserved so the
# emitted act_func_set_id still matches act_info.json.
_orig_gat = bacc.get_activation_tables


def _gat_ln_exp_only(arch):
    tabs = _orig_gat(arch)
    return {name: (fns if name == "natural_log_exp_and_others" else set())
            for name, fns in tabs.items()}


bacc.get_activation_tables = _gat_ln_exp_only

B, C, H, W = 4, 128, 64, 64
N = H * W                      # 4096
NH, DH = 4, 32
EPS = 1e-6
NCH = 512                      # free-dim chunk (psum bank)
NJ = N // NCH                  # 8 n-chunks
MC = 128                       # m-chunk (partition tile)
NM = N // MC                   # 32 m-chunks
SCALE = DH ** -0.5


def build_nc(reps: int = 1, debug: bool = False):
    nc = bacc.Bacc("TRN2", target_bir_lowering=False)
    d_x = nc.dram_tensor("x", [C, N], F32R, kind="ExternalInput")
    # wqq | wkk | wv packed into one DMA (f32r == f32 bit layout)
    d_wb = nc.dram_tensor("wb", [C, 320], F32R, kind="ExternalInput")
    # bqq | bkk | res packed columns
    d_bias = nc.dram_tensor("bias", [128, 3], F32, kind="ExternalInput")
    d_pw = nc.dram_tensor("pw", [65, C], F32R, kind="ExternalInput")  # projT + bias row
    d_out = nc.dram_tensor("out", [C, N], F32, kind="ExternalOutput")
    if debug:
        d_dbg_h = nc.dram_tensor("dbg_h", [64, N], F32, kind="ExternalOutput")
        d_dbg_r = nc.dram_tensor("dbg_r", [2, N], F32, kind="ExternalOutput")
        d_dbg_q = nc.dram_tensor("dbg_q", [C, N], F32R, kind="ExternalOutput")
        d_dbg_k = nc.dram_tensor("dbg_k", [C, N], F32R, kind="ExternalOutput")
        d_dbg_v = nc.dram_tensor("dbg_v", [C, NM * 68], F32, kind="ExternalOutput")

    with tile.TileContext(nc) as tc:
        with tc.tile_pool(name="persist", bufs=1) as P:
            x_sb = P.tile([C, N], F32R, tag="x_sb")
            qq2 = P.tile([C, N], F32R, tag="qq2")
            kk2 = P.tile([C, N], F32R, tag="kk2")
            vta = P.tile([C, NM, 68], BF16, tag="vta")   # per m-chunk: [v0|1|pad|v1|1|pad]
            w_b = P.tile([C, 320], F32R, tag="w_b")
            wr_qq = w_b[:, 0:128]
            wr_kk = w_b[:, 128:256]
            wr_v = w_b[:, 256:320]
            bias3 = P.tile([128, 3], F32, tag="bias3")
            b_qq = bias3[:, 0:1]
            b_kk = bias3[:, 1:2]
            res_c = bias3[:, 2:3]
            wr_p = P.tile([65, C], F32R, tag="wr_p")
            eps_c = P.tile([C, 1], F32, tag="eps_c")
            ones_nm = P.tile([C, NM], F32, tag="ones_nm")
            ones_row = P.tile([1, NCH], F32, tag="ones_row")
            hn_a = P.tile([65, NCH], F32R, tag="hn_a")   # hn + ones row (proj bias)
            hn_b = P.tile([65, NCH], F32R, tag="hn_b")
            hbuf = [P.tile([64, NCH], F32, tag=f"hbuf{j}", name=f"hbuf{j}") for j in range(NJ)]
            rec2 = [P.tile([1, 2 * NCH], F32, tag=f"rec2_{j}", name=f"rec2_{j}") for j in range(NJ)]

            # x chunks 0/1 first: they head the dependency chain; weights are
            # only needed ~10us in
            nc.sync.dma_start(out=x_sb[:, 0:NCH], in_=d_x.ap()[:, 0:NCH])
            nc.sync.dma_start(out=x_sb[:, NCH:2 * NCH],
                              in_=d_x.ap()[:, NCH:2 * NCH])
            nc.sync.dma_start(out=w_b, in_=d_wb.ap())
            nc.sync.dma_start(out=bias3, in_=d_bias.ap())
            nc.sync.dma_start(out=wr_p, in_=d_pw.ap())
            nc.vector.memset(eps_c, EPS)
            nc.vector.memset(ones_nm, 1.0)
            nc.vector.memset(ones_row, 1.0)
            nc.vector.tensor_copy(out=hn_a[64:65, :], in_=ones_row)
            nc.vector.tensor_copy(out=hn_b[64:65, :], in_=ones_row)
            nc.vector.tensor_copy(out=vta[:, :, 32:33], in_=ones_nm)
            nc.vector.tensor_copy(out=vta[:, :, 66:67], in_=ones_nm)

            for rep in range(reps):
                with tc.tile_pool(name="stats", bufs=3) as SP, \
                     tc.tile_pool(name="spool", bufs=2, space="PSUM") as SPOOL, \
                     tc.tile_pool(name="pvpool", bufs=1, space="PSUM") as PVP, \
                     tc.tile_pool(name="pjpool", bufs=1, space="PSUM") as PJP, \
                     tc.tile_pool(name="ptpool", bufs=5) as PTP, \
                     tc.tile_pool(name="opool", bufs=2) as OPO, \
                     tc.tile_pool(name="rpool", bufs=1) as RPO:

                    rbcs = {}
                    s0s, invs = {}, {}

                    # ---------- per-chunk LN stats (Pool + DVE + ACT) ---------
                    def dma_chunk(j):
                        js = slice(j * NCH, (j + 1) * NCH)
                        nc.sync.dma_start(out=x_sb[:, js], in_=d_x.ap()[:, js])

                    s1s = {}

                    def pool_stats_chunk(j):
                        js = slice(j * NCH, (j + 1) * NCH)
                        x2 = SP.tile([C, NCH], F32R, tag="x2", name=f"x2_{j}")
                        nc.gpsimd.tensor_mul(x2, x_sb[:, js], x_sb[:, js])
                        s0 = SP.tile([C, NCH], F32, tag="s0", name=f"s0_{j}")
                        s1 = SP.tile([C, NCH], F32, tag="s1", name=f"s1_{j}")
                        nc.gpsimd.partition_all_reduce(s0, x_sb[:, js], C,
                                                       bass_isa.ReduceOp.add)
                        nc.gpsimd.partition_all_reduce(s1, x2, C,
                                                       bass_isa.ReduceOp.add)
                        s0s[j], s1s[j] = s0, s1

                    def stats_chunk(j):
                        s0, s1 = s0s[j], s1s[j]
                        msq = SP.tile([C, NCH], F32, tag="msq", name=f"msq_{j}")
                        nc.vector.scalar_tensor_tensor(out=msq, in0=s0,
                                                       scalar=1.0 / (C * C),
                                                       in1=s0,
                                                       op0=OP.mult, op1=OP.mult)
                        var = SP.tile([C, NCH], F32, tag="var", name=f"var_{j}")
                        nc.vector.scalar_tensor_tensor(out=var, in0=s1,
                                                       scalar=1.0 / C, in1=msq,
                                                       op0=OP.mult,
                                                       op1=OP.subtract)
                        # rsqrt via exp(-0.5*ln(var+eps)): Ln+Exp share one ACT
                        # table set, so no table reloads against attention exps
                        lnv = SP.tile([C, NCH], F32, tag="lnv", name=f"lnv_{j}")
                        nc.scalar.activation(out=lnv, in_=var, func=AF.Ln,
                                             bias=eps_c, scale=1.0)
                        inv = SP.tile([C, NCH], F32, tag="inv", name=f"inv_{j}")
                        nc.scalar.activation(out=inv, in_=lnv, func=AF.Exp,
                                             scale=-0.5)
                        invs[j] = inv

                    # ---------- per-chunk xhat + qkv projections --------------
                    def qkv_chunk(j):
                        js = slice(j * NCH, (j + 1) * NCH)
                        cen = SP.tile([C, NCH], F32, tag="cen", name=f"cen_{j}")
                        nc.vector.scalar_tensor_tensor(out=cen, in0=s0s[j],
                                                       scalar=-1.0 / C,
                                                       in1=x_sb[:, js],
                                                       op0=OP.mult, op1=OP.add)
                        xhat = SP.tile([C, NCH], F32R, tag="xh", name=f"xh_{j}")
                        nc.vector.tensor_mul(xhat, cen, invs[j])
                        # k first (feeds the streamed chunk-0 attention), then
                        # v, then q
                        qkp = SPOOL.tile([C, 2, NCH], F32, tag="sg", name=f"qkp{j}")
                        nc.tensor.matmul(qkp[:, 1, :], wr_kk, xhat,
                                         start=True, stop=True)
                        nc.vector.tensor_scalar(out=kk2[:, js], in0=qkp[:, 1, :],
                                                scalar1=b_kk,
                                                scalar2=None, op0=OP.add)
                        vpq = SPOOL.tile([C, 4, 64], F32, tag="sg", name=f"vpq{j}")
                        for mq in range(4):
                            mc = 4 * j + mq
                            nc.tensor.matmul(vpq[:, mq, :],
                                             xhat[:, mq * MC:(mq + 1) * MC],
                                             wr_v, start=True, stop=True)
                            vdst = vta[:, mc, 0:68].rearrange(
                                "p (a b) -> p a b", a=2)[:, :, 0:32]
                            vsrc = vpq[:, mq, :].rearrange("p (a b) -> p a b", a=2)
                            nc.vector.tensor_copy(out=vdst, in_=vsrc)
                        nc.tensor.matmul(qkp[:, 0, :], wr_qq, xhat,
                                         start=True, stop=True)
                        nc.vector.tensor_scalar(out=qq2[:, js], in0=qkp[:, 0, :],
                                                scalar1=b_qq,
                                                scalar2=None, op0=OP.add)

                    # ---------- attention machinery ---------------------------
                    pairs = [(i % 2, i // 2) for i in range(2 * NM)]
                    NG = (2 * NM + 2) // 3          # 22 groups per n-chunk
                    PVLAG = 4                       # PV issue lags S/exp issue

                    # pv accumulators: both heads packed into ONE psum bank,
                    # head0 at partitions 0..32, head1 at 64..96 (the matmul
                    # tile_position column offset must be 0 or 64)
                    pvt = {}                        # chunk -> pv tile
                    pend = []                       # pending PV emits

                    def pv_tile(j):
                        if j not in pvt:
                            pvt[j] = PVP.tile([97, NCH], F32, tag="pv",
                                              name=f"pv_{j}")
                        return pvt[j]

                    def emit_pv():
                        j, gi, pt, grp = pend.pop(0)
                        pv = pv_tile(j)
                        for i, (h, mc) in enumerate(grp):
                            pi = 3 * gi + i
                            vcols = slice(34 * h, 34 * h + 33)
                            dst = pv[0:33, :] if h == 0 else pv[64:97, :]
                            nc.tensor.matmul(dst, vta[:, mc, vcols],
                                             pt[:, i * NCH:(i + 1) * NCH],
                                             start=(pi == h),
                                             stop=(pi == 2 * NM - 2 + h))
                        if gi == NG - 1:
                            attn_end(j)

                    def attn_group(j, gi):
                        js = slice(j * NCH, (j + 1) * NCH)
                        g0 = 3 * gi
                        grp = pairs[g0:g0 + 3]
                        sg = SPOOL.tile([C, 3 * NCH], F32, tag="sg",
                                        name=f"sg{j}_{gi}")
                        seen = {0: 0, 1: 0}
                        for i, (h, mc) in enumerate(grp):
                            rg = h + 2 * seen[h]     # row-group 0..3
                            seen[h] += 1
                            ms = slice(mc * MC, (mc + 1) * MC)
                            rs = slice(rg * 32, (rg + 1) * 32)
                            nc.tensor.matmul(sg[:, i * NCH:(i + 1) * NCH],
                                             kk2[rs, ms], qq2[rs, js],
                                             start=True, stop=True,
                                             tile_position=(rg * 32, 0))
                        pt = PTP.tile([C, 3 * NCH], BF16, tag="pt",
                                      name=f"pt{j}_{gi}")
                        nw = len(grp) * NCH
                        nc.scalar.activation(out=pt[:, 0:nw], in_=sg[:, 0:nw],
                                             func=AF.Exp, scale=SCALE)
                        pend.append((j, gi, pt, grp))
                        while len(pend) > PVLAG:
                            emit_pv()

                    def attn_end(j):
                        if j == NJ - 1:
                            drain_split(j)
                            return
                        pv = pvt[j]
                        nc.vector.tensor_copy(out=hbuf[j][0:32, :],
                                              in_=pv[0:32, :])
                        nc.vector.tensor_copy(out=hbuf[j][32:64, :],
                                              in_=pv[64:96, :])
                        nc.vector.reciprocal(out=rec2[j][:, 0:NCH],
                                             in_=pv[32:33, :])
                        nc.vector.reciprocal(out=rec2[j][:, NCH:2 * NCH],
                                             in_=pv[96:97, :])
                        # partition_broadcast ignores the out partition offset,
                        # so broadcast both heads' recip rows side-by-side and
                        # pick the right half per head in tail_a
                        rbc = RPO.tile([64, 2 * NCH], F32, tag="rbc",
                                       name=f"rbc{j}")
                        nc.gpsimd.partition_broadcast(rbc, rec2[j])
                        rbcs[j] = rbc

                    def drain_split(j):
                        # last chunk: no later work hides the tail, so run it
                        # in two column halves pipelined across DVE/Pool/PE,
                        # reading h straight from PSUM (no hbuf staging)
                        pv = pvt[j]
                        hn = hn_a if j % 2 == 0 else hn_b
                        rbc = RPO.tile([64, 2 * NCH], F32, tag="rbc",
                                       name=f"rbc{j}")
                        pj = PJP.tile([C, NCH], F32, tag="pj", name=f"pj{j}")
                        HC = NCH // 2
                        halves = []
                        for half in range(2):
                            hs = slice(half * HC, half * HC + HC)
                            gs = slice(j * NCH + half * HC,
                                       j * NCH + half * HC + HC)
                            base = half * NCH
                            halves.append((hs, gs, base))
                            nc.vector.reciprocal(
                                out=rec2[j][:, base:base + HC],
                                in_=pv[32:33, hs])
                            nc.vector.reciprocal(
                                out=rec2[j][:, base + HC:base + NCH],
                                in_=pv[96:97, hs])
                        for hs, gs, base in halves:
                            nc.gpsimd.partition_broadcast(
                                rbc[:, base:base + NCH],
                                rec2[j][:, base:base + NCH])
                        for hs, gs, base in halves:
                            nc.vector.tensor_mul(hn[0:32, hs], pv[0:32, hs],
                                                 rbc[0:32, base:base + HC])
                            nc.vector.tensor_mul(hn[32:64, hs], pv[64:96, hs],
                                                 rbc[32:64, base + HC:base + NCH])
                            nc.tensor.matmul(pj[:, hs], wr_p, hn[:, hs],
                                             start=True, stop=True)
                        for half, (hs, gs, base) in enumerate(halves):
                            ot = OPO.tile([C, HC], F32, tag="oth",
                                          name=f"ot{j}_{half}")
                            nc.vector.scalar_tensor_tensor(
                                out=ot, in0=x_sb[:, gs], scalar=res_c,
                                in1=pj[:, hs], op0=OP.mult, op1=OP.add)
                            nc.sync.dma_start(out=d_out.ap()[:, gs], in_=ot)

                    def tail_a(j):
                        hn = hn_a if j % 2 == 0 else hn_b
                        rbc = rbcs[j]
                        nc.vector.tensor_mul(hn[0:32, :], hbuf[j][0:32, :],
                                             rbc[0:32, 0:NCH])
                        nc.vector.tensor_mul(hn[32:64, :], hbuf[j][32:64, :],
                                             rbc[32:64, NCH:2 * NCH])

                    def tail_b(j):
                        js = slice(j * NCH, (j + 1) * NCH)
                        hn = hn_a if j % 2 == 0 else hn_b
                        pj = PJP.tile([C, NCH], F32, tag="pj", name=f"pj{j}")
                        nc.tensor.matmul(pj, wr_p, hn, start=True, stop=True)
                        ot = OPO.tile([C, NCH], F32, tag="ot", name=f"ot{j}")
                        nc.vector.scalar_tensor_tensor(out=ot, in0=x_sb[:, js],
                                                       scalar=res_c, in1=pj,
                                                       op0=OP.mult, op1=OP.add)
                        nc.sync.dma_start(out=d_out.ap()[:, js], in_=ot)

                    # ---------- schedule --------------------------------------
                    next_g = [0]
                    # dependency-free matmuls into the (startup-idle) pj bank:
                    # they keep the PE p-state ramp hot while real matmuls
                    # wait on the DVE qkv chain, so the streamed S matmuls run
                    # at full clock instead of the 2-4x derated cold clock
                    fil = PJP.tile([C, NCH], F32, tag="pj", name="fil")
                    fdum = SP.tile([C, 64], BF16, tag="fdum", name="fdum")
                    nc.vector.memset(fdum, 0.5)

                    def flood(k):
                        for _ in range(k):
                            nc.tensor.matmul(fil[0:64, 0:64], fdum, fdum,
                                             start=True, stop=True)

                    def stream_j0(c):
                        # emit chunk-0 groups whose kk/vta m-chunks are
                        # already written (program order defines the dep
                        # graph -- emitting ahead of the writer would read
                        # stale data)
                        while next_g[0] < NG and \
                                min(3 * next_g[0] + 2, 2 * NM - 1) // 2 <= 4 * c + 3:
                            attn_group(0, next_g[0])
                            next_g[0] += 1

                    # software pipeline: Pool reductions run two chunks ahead,
                    # the DVE var + ACT ln/exp one chunk ahead, and qkv+stream
                    # lead each iteration so chunk j's critical chain is never
                    # queued behind later chunks' stats.
                    dma_chunk(2)
                    pool_stats_chunk(0)
                    pool_stats_chunk(1)
                    stats_chunk(0)
                    for j in range(NJ):
                        if j + 3 < NJ:
                            dma_chunk(j + 3)
                        if j + 1 < NJ:
                            stats_chunk(j + 1)
                        qkv_chunk(j)
                        flood(10)
                        stream_j0(j)
                        flood(10)
                        if j + 2 < NJ:
                            pool_stats_chunk(j + 2)
                    for j in range(1, NJ):
                        for gi in range(NG):
                            if gi == 5:
                                tail_a(j - 1)
                            if gi == 9:
                                tail_b(j - 1)
                            attn_group(j, gi)
                    while pend:
                        emit_pv()
                    if debug:
                        for j in range(NJ):
                            js = slice(j * NCH, (j + 1) * NCH)
                            nc.sync.dma_start(out=d_dbg_h.ap()[:, js], in_=hbuf[j])
                            nc.sync.dma_start(out=d_dbg_r.ap()[0:1, js],
                                              in_=rec2[j][:, 0:NCH])
                            nc.sync.dma_start(out=d_dbg_r.ap()[1:2, js],
                                              in_=rec2[j][:, NCH:2 * NCH])
                        nc.sync.dma_start(out=d_dbg_q.ap(), in_=qq2)
                        nc.sync.dma_start(out=d_dbg_k.ap(), in_=kk2)
                        vf = OPO.tile([C, NM * 68], F32, tag="vf", name="vf")
                        nc.vector.tensor_copy(out=vf, in_=vta.rearrange("p a b -> p (a b)"))
                        nc.sync.dma_start(out=d_dbg_v.ap(), in_=vf)
    nc.compile()
    return nc


def _prep_inputs(x, norm_w, norm_b, qkv_w, qkv_b, proj_w, proj_b):
    """Host-side fold + per-core slicing. Returns list of 8 in_maps."""
    xf = np.ascontiguousarray(x.reshape(B, C, N), dtype=np.float32)
    qkv_wf = (qkv_w * norm_w[None, :]).astype(np.float32)
    qkv_bf = (qkv_b + qkv_w @ norm_b).astype(np.float32)
    in_maps = []
    for core in range(8):
        b, hp = core // 2, core % 2
        h0, h1 = 2 * hp, 2 * hp + 1
        qrows = list(range(h0 * DH, h0 * DH + DH)) + list(range(h1 * DH, h1 * DH + DH))
        krows = [C + r for r in qrows]
        vrows = [2 * C + r for r in qrows]
        qrows2 = qrows + qrows                           # duplicated for row-packing
        krows2 = krows + krows
        wb = np.empty((C, 320), np.float32)
        wb[:, 0:128] = qkv_wf[qrows2, :].T               # wqq
        wb[:, 128:256] = qkv_wf[krows2, :].T             # wkk
        wb[:, 256:320] = qkv_wf[vrows, :].T              # wv
        bias = np.zeros((128, 3), np.float32)
        bias[:, 0] = qkv_bf[qrows2]                      # bqq
        bias[:, 1] = qkv_bf[krows2]                      # bkk
        bias[:, 2] = 1.0 if hp == 0 else 0.0             # residual scale
        bv = qkv_bf[vrows]                               # [64], this core's heads
        cols = qrows
        pw = np.zeros((65, C), np.float32)
        pw[0:64, :] = proj_w[:, cols].T
        # v-bias folded past the softmax: h/den + bv, so proj bias picks up
        # proj_w[:, cols] @ bv per core (plus proj_b once, on the hp==0 core)
        pw[64, :] = proj_w[:, cols] @ bv
        if hp == 0:
            pw[64, :] += proj_b
        in_maps.append({
            "x": np.ascontiguousarray(xf[b]), "wb": wb, "bias": bias, "pw": pw,
        })
    return in_maps


_NC_CACHE = None


def kernel(x, norm_w, norm_b, qkv_w, qkv_b, proj_w, proj_b, **extra):
    global _NC_CACHE
    x = np.asarray(x, dtype=np.float32)
    in_maps = _prep_inputs(x, np.asarray(norm_w), np.asarray(norm_b),
                           np.asarray(qkv_w), np.asarray(qkv_b),
                           np.asarray(proj_w), np.asarray(proj_b))
    if _NC_CACHE is None:
        _NC_CACHE = build_nc()
    res = run_bass_kernel_spmd(_NC_CACHE, in_maps, core_ids=list(range(8)))
    parts = [res.results[i]["out"] for i in range(8)]
    out = np.empty((B, C, N), np.float32)
    for b in range(B):
        out[b] = parts[2 * b] + parts[2 * b + 1]
    return out.reshape(B, C, H, W)


if __name__ == "__main__":
    rng = np.random.default_rng(0)
    x = rng.standard_normal((B, C, H, W)).astype(np.float32)
    nw = np.ones(C, np.float32)
    nb = np.zeros(C, np.float32)
    qw = (rng.standard_normal((3 * C, C)) / np.sqrt(C)).astype(np.float32)
    qb = np.zeros(3 * C, np.float32)
    pw = (rng.standard_normal((C, C)) / np.sqrt(C)).astype(np.float32)
    pb = np.zeros(C, np.float32)
    got = kernel(x, nw, nb, qw, qb, pw, pb)
    print("kernel ran, shape", got.shape)
